# revision 20
# baseline (speedup 1.0000x reference)
"""MsPoE Llama attention on 8 TRN2 NeuronCores (tensor-parallel over heads).

Strategy (v2: single launch)
----------------------------
The reference's head-ordering statistic only needs the LAST pre-RoPE
attention row: srow_h = q_last_h . k_h[s]. By associativity,
srow_h = hs @ (Wk_h^T (Wq_h hs[-1])) — ~0.5 GFLOP, computed on the
host in float64 BEFORE launching (verified to reproduce the reference
head_order exactly: min margin to the 3*avg threshold is ~1e-5 vs
~6e-7 fp32-path noise). With head_order known up-front, the permuted
per-head RoPE cos/sin caches become plain inputs and the whole module
runs in ONE device launch with q/k/v resident in SBUF:

  Per core (4 heads), all matmul operands bf16 (1 PE cycle/row — the
  same rate as fp32r — but half the DMA/SBUF, and no f32->f32r
  conversion casts at all):

  1. QK pass: stream hsT once, accumulate q/k head-blocks in PSUM
     (8 banks), apply RoPE directly out of PSUM on DVE (+gpsimd for
     one swap-half) into resident rq/rk [128, 4, 2048] bf16 tiles.
     rotate_half's partition swap is two half-partition reads at
     offset 64/0; the sin sign flip is pre-applied on the host (shat).
  2. V pass: stream hsT again (wq+wk+wv cannot be resident at once),
     natural-layout V into resident v_m [128, 16, 512] bf16.
  3. Attention per (qb outer, head inner): scores^T = rk_chunk^T rq
     per 128-key tile, exp on ACT (bf16 out), softmax denominator via
     ones-matmul accumulation, unnormalized AV accumulation; 1/z is
     partition-broadcast with a tiny PE outer-product (no DRAM
     roundtrip); attnT normalized in place (DVE).
  4. o_proj for qb is emitted right after qb's heads finish, so its
     matmuls overlap the next qb's attention chain; partials oT
     [4096, 2048] f32 stream out per 512-column block.

  Host: sum the 8 o_proj partials (f64) -> [1, 2048, 4096].
"""

import os
import sys

import numpy as np

for _p in ("/opt/trn_rl_repo", "/root/.axon_site/_ro/trn_rl_repo"):
    if os.path.isdir(_p) and _p not in sys.path:
        sys.path.append(_p)

import concourse.bass as bass  # noqa: E402
import concourse.tile as tile  # noqa: E402
from concourse import bacc, mybir  # noqa: E402
from concourse import bass_utils  # noqa: E402

import ml_dtypes  # noqa: E402

F32 = mybir.dt.float32
BF16 = mybir.dt.bfloat16
F32R = mybir.dt.float32r
NPBF16 = ml_dtypes.bfloat16

B, S, HID, H, D = 1, 2048, 4096, 32, 128
NCORES, HPC = 8, 4          # cores, heads per core
JC = HPC * D                # 512: per-core projection width
KT = HID // 128             # 32 contraction tiles
SB = S // 512               # 4 sequence blocks
EB = 4                      # e-tiles per hs-stream DMA
BASE, MIN_R, MAX_R = 10000.0, 1.0, 3.0
SCALE = 1.0 / float(np.sqrt(D))
NEGM = -1.0e35              # additive causal mask value (exp -> 0)

_CACHE = {}
TRACE = False          # set True (e.g. from test.py) to profile the launch
LAST_PROFILE = {}      # filled with BassKernelResults when TRACE is on


def build():
    nc = bacc.Bacc("TRN2", target_bir_lowering=False, debug=False, num_devices=NCORES)
    hsT = nc.dram_tensor("hsT", [HID, S], BF16, kind="ExternalInput").ap()
    wqkT = nc.dram_tensor("wqkT", [HID, 2 * JC], BF16, kind="ExternalInput").ap()
    wvT = nc.dram_tensor("wvT", [HID, JC], BF16, kind="ExternalInput").ap()
    woT = nc.dram_tensor("woT", [JC, HID], BF16, kind="ExternalInput").ap()
    cosT = nc.dram_tensor("cosT", [JC, S], BF16, kind="ExternalInput").ap()
    shatT = nc.dram_tensor("shatT", [JC, S], BF16, kind="ExternalInput").ap()
    masks = nc.dram_tensor("masks", [128, 4 * 512], F32, kind="ExternalInput").ap()
    oT = nc.dram_tensor("oT", [HID, S], BF16, kind="ExternalOutput").ap()

    hsT_b = hsT.rearrange("(eb g p) s -> p eb g s", p=128, g=EB)   # [128, 8, EB, S]
    wqkT_b = wqkT.rearrange("(eb g p) j -> p eb g j", p=128, g=EB)  # [128, 8, EB, 2JC]
    wvT_b = wvT.rearrange("(kt p) j -> p kt j", p=128)             # [128, 32, JC]
    woT_b = woT.rearrange("(jt p) e -> p jt e", p=128)             # [128, 4, HID]
    cosT_b = cosT.rearrange("(h p) s -> p h s", p=128)             # [128, 4, S]
    shatT_b = shatT.rearrange("(h p) s -> p h s", p=128)
    oT_b = oT.rearrange("(et p) s -> p et s", p=128)               # [128, 32, S]
    NB = KT // EB

    with tile.TileContext(nc) as tc:
        with (
            tc.tile_pool(name="wres", bufs=2) as wres,       # wv_m, wo_m resident
            tc.tile_pool(name="wst", bufs=4) as wst,         # wq/wk streamed chunks
            tc.tile_pool(name="trig", bufs=4) as trig,       # cos/shat per-sb stream
            tc.tile_pool(name="big", bufs=4) as big,         # rq/rk/v/attnT resident
            tc.tile_pool(name="hpool", bufs=2) as hpool,     # hs stream tiles
            tc.tile_pool(name="rtmp", bufs=4) as rtmp,       # RoPE f32 temps
            tc.tile_pool(name="expp", bufs=3) as expp,       # exp tiles bf16
            tc.tile_pool(name="small", bufs=1) as small,
            tc.tile_pool(name="zp", bufs=4) as zp,           # 1/z rows
            tc.tile_pool(name="outp", bufs=4) as outp,
            tc.tile_pool(name="ps", bufs=8, space="PSUM") as ps,
        ):
            rq = big.tile([128, HPC, S], BF16, tag="big", name="rq")
            rk = big.tile([128, HPC, S], BF16, tag="big", name="rk")
            wv_m = wres.tile([128, KT, JC], BF16, tag="w", name="wv")
            wo_m = wres.tile([128, HPC, HID], BF16, tag="w", name="wo")

            # ---- phase 1: QK projections + fused RoPE ----
            # wq/wk stream in eb-sized chunks alongside hs, so the PE can
            # start ~3 DMAs in; wv/wo resident loads trickle in one chunk
            # per sb and are ready by phase 2.
            for sb in range(SB):
                ss = slice(sb * 512, (sb + 1) * 512)
                ps_q = [ps.tile([128, 512], F32, tag="ps", name=f"psq{sb}_{i}")
                        for i in range(HPC)]
                ps_k = [ps.tile([128, 512], F32, tag="ps", name=f"psk{sb}_{i}")
                        for i in range(HPC)]
                cos_t = trig.tile([128, HPC, 512], BF16, tag="t", name=f"cos{sb}")
                shat_t = trig.tile([128, HPC, 512], BF16, tag="t", name=f"shat{sb}")
                for eb in range(NB):
                    hst = hpool.tile([128, EB, 512], BF16, tag="h")
                    nc.sync.dma_start(hst, hsT_b[:, eb, :, ss])
                    wqk = wst.tile([128, EB, 2 * JC], BF16, tag="w",
                                   name=f"wqk{sb}_{eb}", bufs=2)
                    nc.sync.dma_start(wqk, wqkT_b[:, eb])
                    if eb == 0:
                        nc.sync.dma_start(cos_t, cosT_b[:, :, ss])
                        nc.sync.dma_start(shat_t, shatT_b[:, :, ss])
                    for g in range(EB):
                        e = eb * EB + g
                        for jt in range(HPC):
                            js = slice(jt * 128, (jt + 1) * 128)
                            nc.tensor.matmul(
                                ps_q[jt], wqk[:, g, js], hst[:, g],
                                start=(e == 0), stop=(e == KT - 1),
                            )
                            nc.tensor.matmul(
                                ps_k[jt], wqk[:, g, 512 + jt * 128: 512 + (jt + 1) * 128],
                                hst[:, g],
                                start=(e == 0), stop=(e == KT - 1),
                            )
                # prefetch one quarter of wv/wo per sb
                pcs = slice(sb * 8, (sb + 1) * 8)
                nc.sync.dma_start(wv_m[:, pcs], wvT_b[:, pcs])
                nc.sync.dma_start(wo_m[:, :, sb * 1024:(sb + 1) * 1024],
                                  woT_b[:, :, sb * 1024:(sb + 1) * 1024])
                # RoPE: dst = qf*cos + rot_half(qf)*sin. shat holds the
                # HALF-SWAPPED signed sin (host-prepared), so both swap
                # multiplies have partition-ALIGNED inputs and only the
                # OUTPUT is partition-shifted (verified exact on HW).
                for ps_list, dst in ((ps_q, rq), (ps_k, rk)):
                    for jt in range(HPC):
                        p = ps_list[jt]
                        qf = rtmp.tile([128, 512], F32, tag="rt")
                        nc.scalar.copy(qf, p)   # frees the PSUM bank early
                        tmp = rtmp.tile([128, 512], F32, tag="rt")
                        nc.gpsimd.tensor_mul(
                            tmp[64:128], qf[0:64], shat_t[0:64, jt]
                        )
                        nc.vector.tensor_mul(
                            tmp[0:64], qf[64:128], shat_t[64:128, jt]
                        )
                        t2 = rtmp.tile([128, 512], F32, tag="rt")
                        nc.vector.tensor_mul(t2, qf, cos_t[:, jt])
                        with nc.allow_low_precision(reason="rope bf16 store"):
                            nc.vector.tensor_add(dst[:, jt, ss], t2, tmp)

            # ---- constants for attention (loaded behind phase 2's stream) ----
            masks_sb = small.tile([128, 4, 512], F32)
            nc.sync.dma_start(masks_sb, masks.rearrange("p (r j) -> p r j", r=4))
            onesf = small.tile([128, 1], F32)
            nc.vector.memset(onesf, 1.0)
            ones_bf = small.tile([128, 1], BF16)
            nc.vector.tensor_copy(ones_bf, onesf)
            onesf_r = small.tile([1, 128], F32)
            nc.vector.memset(onesf_r, 1.0)
            ones_row = small.tile([1, 128], F32R)
            nc.vector.tensor_copy(ones_row, onesf_r)

            # ---- phase 2: V projection (natural layout) ----
            v_m = big.tile([128, S // 128, JC], BF16, tag="big", name="v")
            for sb in range(SB):
                ss = slice(sb * 512, (sb + 1) * 512)
                ps_v = [ps.tile([128, 512], F32, tag="ps", name=f"psv{sb}_{i}")
                        for i in range(4)]
                for eb in range(NB):
                    hst = hpool.tile([128, EB, 512], BF16, tag="h")
                    nc.sync.dma_start(hst, hsT_b[:, eb, :, ss])
                    for g in range(EB):
                        e = eb * EB + g
                        for t4 in range(4):
                            cs = slice(t4 * 128, (t4 + 1) * 128)
                            nc.tensor.matmul(
                                ps_v[t4], hst[:, g, cs], wv_m[:, e],
                                start=(e == 0), stop=(e == KT - 1),
                            )
                for t4 in range(4):
                    nc.scalar.copy(v_m[:, sb * 4 + t4], ps_v[t4])

            # ---- phase 3+4: attention (qb outer) + interleaved o_proj ----
            attnT = big.tile([128, HPC * SB, 512], BF16, tag="big", name="attnT")
            oo_flip = 0

            def emit_oproj(oqb, et):
                nonlocal oo_flip
                oqs = slice(oqb * 512, (oqb + 1) * 512)
                ps_oo = ps.tile([128, 512], F32, tag="ps",
                                name=f"poo{oqb}_{et}")
                for jt in range(HPC):
                    nc.tensor.matmul(
                        ps_oo, wo_m[:, jt, et * 128: (et + 1) * 128],
                        attnT[:, jt * SB + oqb],
                        start=(jt == 0), stop=(jt == HPC - 1),
                    )
                oo = outp.tile([128, 512], BF16, tag="oo", bufs=5)
                if oo_flip % 2 == 0:
                    nc.scalar.copy(oo, ps_oo)
                else:
                    nc.vector.tensor_copy(oo, ps_oo)
                oo_flip += 1
                nc.sync.dma_start(oT_b[:, et, oqs], oo)

            # o_proj for qb-1 is interleaved INTO qb's attention kt-loop:
            # its matmuls fill the exp-chain stalls, and attention's ACT
            # work hides under o_proj's solid PE blocks.
            pending = []
            for qb in range(SB):
                qs = slice(qb * 512, (qb + 1) * 512)
                nkt = 4 * qb + 4
                zrs = []
                pos_ = []
                for h in range(HPC):
                    i16 = h * SB + qb
                    ps_o = ps.tile([128, 512], F32, tag="ps", name=f"pso{qb}_{h}")
                    ps_z = ps.tile([1, 512], F32, tag="ps", name=f"psz{qb}_{h}")

                    def score_exp(kt):
                        # diagonal blocks: columns j < 128*r are fully
                        # masked -> skip them (w = valid width)
                        r = kt - 4 * qb
                        j0 = 128 * r if r > 0 else 0
                        w = 512 - j0
                        qsw_ = slice(qb * 512 + j0, (qb + 1) * 512)
                        ps_s = ps.tile([128, w], F32, tag="ps",
                                       name=f"pss{qb}_{h}_{kt}")
                        nc.tensor.matmul(
                            ps_s, rk[:, h, kt * 128: (kt + 1) * 128],
                            rq[:, h, qsw_], start=True, stop=True,
                        )
                        if r >= 0:
                            nc.vector.tensor_add(
                                ps_s, ps_s, masks_sb[:, r, j0:512]
                            )
                        ext = expp.tile([128, w], BF16, tag="exp")
                        nc.scalar.activation(
                            ext, ps_s, mybir.ActivationFunctionType.Exp,
                            scale=SCALE,
                        )
                        return ext, j0

                    # software pipeline: score/exp one kt ahead of z/AV, so
                    # the PE never waits on the mask+exp latency chain
                    nxt = score_exp(0)
                    for kt in range(nkt):
                        ext, j0 = nxt
                        if kt + 1 < nkt:
                            nxt = score_exp(kt + 1)
                        nc.tensor.matmul(
                            ps_z[:, j0:512], ones_bf, ext,
                            start=(kt == 0), stop=(kt == nkt - 1),
                        )
                        nc.tensor.matmul(
                            ps_o[:, j0:512],
                            v_m[:, kt, h * 128: (h + 1) * 128], ext,
                            start=(kt == 0), stop=(kt == nkt - 1),
                        )
                        if pending:
                            emit_oproj(*pending.pop(0))
                    # drain unnormalized rows (frees ps_o), 1/z row (frees ps_z)
                    nc.vector.tensor_copy(attnT[:, i16], ps_o)
                    zf = zp.tile([1, 512], F32, tag="zf", bufs=2)
                    nc.vector.reciprocal_approx_fast(zf, ps_z)
                    zr = zp.tile([1, 512], F32R, tag="zr")
                    nc.vector.tensor_copy(zr, zf)
                    zrs.append(zr)
                    pos_.append(i16)
                # normalize: broadcast 1/z across partitions on the PE
                for h in range(HPC):
                    zb = ps.tile([128, 512], F32, tag="ps", name=f"zb{qb}_{h}")
                    nc.tensor.matmul(zb, ones_row, zrs[h], start=True, stop=True)
                    i16 = pos_[h]
                    with nc.allow_low_precision(reason="attn normalize bf16"):
                        nc.vector.tensor_tensor(
                            attnT[:, i16], attnT[:, i16], zb,
                            op=mybir.AluOpType.mult,
                        )
                for job in pending:      # leftovers from qb-1
                    emit_oproj(*job)
                pending = [(qb, et) for et in range(KT)]
            for job in pending:              # final query block's o_proj
                emit_oproj(*job)

    nc.compile()
    return nc


def _get_nc():
    if "S" not in _CACHE:
        _CACHE["S"] = build()
    return _CACHE["S"]


def _causal_mask_templates():
    # masked (NEGM) iff 128*r + p > j for p in [0,128), j in [0,512)
    p = np.arange(128)[:, None]
    j = np.arange(512)[None, :]
    out = np.zeros((128, 4, 512), np.float32)
    for r in range(4):
        out[:, r, :] = np.where(128 * r + p > j, NEGM, 0.0).astype(np.float32)
    return np.ascontiguousarray(out.reshape(128, 4 * 512))


def _rope_cache_np():
    # mirrors reference._rope_cache in float32
    inv_freq = (1.0 / (BASE ** (np.arange(0, D, 2, dtype=np.float32) / np.float32(D)))).astype(np.float32)
    ratio = (MIN_R + (MAX_R - MIN_R) * (np.arange(H, dtype=np.float32) / np.float32(H))).astype(np.float32)
    t = (np.arange(S, dtype=np.float32)[None, :] / ratio[:, None]).astype(np.float32)
    freqs = (t[:, :, None] * inv_freq[None, None, :]).astype(np.float32)
    emb = np.concatenate([freqs, freqs], axis=-1)
    return np.cos(emb).astype(np.float32), np.sin(emb).astype(np.float32)


def _head_order(hs, Wq, Wk):
    """Exact head-outlier ordering from the last pre-RoPE attention row,
    computed in f64 on the host: srow_h = hs @ (Wk_h^T (Wq_h hs[-1]))."""
    hs64 = hs.astype(np.float64)
    q_last = hs64[-1] @ Wq.T.astype(np.float64)                 # [HID]
    Wk64 = Wk.astype(np.float64)
    Wall = np.empty((HID, H), np.float64)
    for h in range(H):
        rows = slice(h * D, (h + 1) * D)
        Wall[:, h] = Wk64[rows, :].T @ q_last[rows]
    srow = (hs64 @ Wall).T                                      # [H, S]
    sc = srow * SCALE
    m = sc.max(axis=-1, keepdims=True)
    e = np.exp(sc - m)
    aw = e / e.sum(axis=-1, keepdims=True)
    avg = aw.mean(axis=-1, keepdims=True)
    cnt = (aw > 3.0 * avg).sum(axis=-1)
    outlier = (-(cnt / np.float32(S))).astype(np.float32)
    return np.argsort(outlier, kind="stable")


def kernel(hidden_states, position_ids, Wq, Wk, Wv, Wo):
    hs = np.asarray(hidden_states, dtype=np.float32)[0]        # [S, HID]
    pos = np.asarray(position_ids).astype(np.int64)[0]         # [S]
    Wq = np.asarray(Wq, dtype=np.float32)
    Wk = np.asarray(Wk, dtype=np.float32)
    Wv = np.asarray(Wv, dtype=np.float32)
    Wo = np.asarray(Wo, dtype=np.float32)

    # ---- host: head order (exact control flow), permuted RoPE caches ----
    head_order = _head_order(hs, Wq, Wk)
    cos, sin = _rope_cache_np()
    cos_o = cos[head_order][:, pos, :]                         # [H, S, D]
    sin_o = sin[head_order][:, pos, :]
    masks = _causal_mask_templates()

    hsT = np.ascontiguousarray(hs.T).astype(NPBF16)            # [HID, S] bf16

    nc = _get_nc()
    in_maps = []
    for c in range(NCORES):
        rows = slice(c * JC, (c + 1) * JC)
        ct = np.ascontiguousarray(
            np.concatenate([cos_o[c * HPC + i].T for i in range(HPC)], axis=0)
        )  # [JC, S]
        # half-swapped signed sin: spre[0:64] = +sin[64:128],
        # spre[64:128] = -sin[0:64] (per head) — see RoPE comment in build()
        st = np.concatenate(
            [
                np.concatenate(
                    [sin_o[c * HPC + i].T[D // 2:], -sin_o[c * HPC + i].T[: D // 2]],
                    axis=0,
                )
                for i in range(HPC)
            ],
            axis=0,
        )
        in_maps.append(
            {
                "hsT": hsT,
                "wqkT": np.ascontiguousarray(
                    np.concatenate([Wq[rows, :].T, Wk[rows, :].T], axis=1)
                ).astype(NPBF16),
                "wvT": np.ascontiguousarray(Wv[rows, :].T).astype(NPBF16),
                "woT": np.ascontiguousarray(Wo[:, rows].T).astype(NPBF16),
                "cosT": ct.astype(NPBF16),
                "shatT": np.ascontiguousarray(st).astype(NPBF16),
                "masks": masks,
            }
        )
    res = bass_utils.run_bass_kernel_spmd(
        nc, in_maps, core_ids=list(range(NCORES)), trace=TRACE
    )
    if TRACE:
        LAST_PROFILE["S"] = res

    # ---- host: unshard (sum o_proj partials) ----
    acc = np.zeros((HID, S), np.float64)
    for c in range(NCORES):
        acc += res.results[c]["oT"].astype(np.float64)
    return np.ascontiguousarray(acc.T)[None, :, :].astype(np.float32)


# revision 22
# speedup vs baseline: 1.0463x; 1.0463x over previous
"""MsPoE Llama attention on 8 TRN2 NeuronCores (tensor-parallel over heads).

Strategy (v2: single launch)
----------------------------
The reference's head-ordering statistic only needs the LAST pre-RoPE
attention row: srow_h = q_last_h . k_h[s]. By associativity,
srow_h = hs @ (Wk_h^T (Wq_h hs[-1])) — ~0.5 GFLOP, computed on the
host in float64 BEFORE launching (verified to reproduce the reference
head_order exactly: min margin to the 3*avg threshold is ~1e-5 vs
~6e-7 fp32-path noise). With head_order known up-front, the permuted
per-head RoPE cos/sin caches become plain inputs and the whole module
runs in ONE device launch with q/k/v resident in SBUF:

  Per core (4 heads), all matmul operands bf16 (1 PE cycle/row — the
  same rate as fp32r — but half the DMA/SBUF, and no f32->f32r
  conversion casts at all):

  1. QK pass: stream hsT once, accumulate q/k head-blocks in PSUM
     (8 banks), apply RoPE directly out of PSUM on DVE (+gpsimd for
     one swap-half) into resident rq/rk [128, 4, 2048] bf16 tiles.
     rotate_half's partition swap is two half-partition reads at
     offset 64/0; the sin sign flip is pre-applied on the host (shat).
  2. V pass: stream hsT again (wq+wk+wv cannot be resident at once),
     natural-layout V into resident v_m [128, 16, 512] bf16.
  3. Attention per (qb outer, head inner): scores^T = rk_chunk^T rq
     per 128-key tile, exp on ACT (bf16 out), softmax denominator via
     ones-matmul accumulation, unnormalized AV accumulation; 1/z is
     partition-broadcast with a tiny PE outer-product (no DRAM
     roundtrip); attnT normalized in place (DVE).
  4. o_proj for qb is emitted right after qb's heads finish, so its
     matmuls overlap the next qb's attention chain; partials oT
     [4096, 2048] f32 stream out per 512-column block.

  Host: sum the 8 o_proj partials (f64) -> [1, 2048, 4096].
"""

import os
import sys

import numpy as np

for _p in ("/opt/trn_rl_repo", "/root/.axon_site/_ro/trn_rl_repo"):
    if os.path.isdir(_p) and _p not in sys.path:
        sys.path.append(_p)

import concourse.bass as bass  # noqa: E402
import concourse.tile as tile  # noqa: E402
from concourse import bacc, mybir  # noqa: E402
from concourse import bass_utils  # noqa: E402

import ml_dtypes  # noqa: E402

F32 = mybir.dt.float32
BF16 = mybir.dt.bfloat16
F32R = mybir.dt.float32r
NPBF16 = ml_dtypes.bfloat16

B, S, HID, H, D = 1, 2048, 4096, 32, 128
NCORES, HPC = 8, 4          # cores, heads per core
JC = HPC * D                # 512: per-core projection width
KT = HID // 128             # 32 contraction tiles
SB = S // 512               # 4 sequence blocks
EB = 4                      # e-tiles per hs-stream DMA
BASE, MIN_R, MAX_R = 10000.0, 1.0, 3.0
SCALE = 1.0 / float(np.sqrt(D))
NEGM = -1.0e35              # additive causal mask value (exp -> 0)

_CACHE = {}
TRACE = False          # set True (e.g. from test.py) to profile the launch
LAST_PROFILE = {}      # filled with BassKernelResults when TRACE is on


def build():
    nc = bacc.Bacc("TRN2", target_bir_lowering=False, debug=False, num_devices=NCORES)
    hsT = nc.dram_tensor("hsT", [HID, S], BF16, kind="ExternalInput").ap()
    wqT = nc.dram_tensor("wqT", [HID, JC], BF16, kind="ExternalInput").ap()
    wkT = nc.dram_tensor("wkT", [HID, JC], BF16, kind="ExternalInput").ap()
    wvT = nc.dram_tensor("wvT", [HID, JC], BF16, kind="ExternalInput").ap()
    woT = nc.dram_tensor("woT", [JC, HID], BF16, kind="ExternalInput").ap()
    cosT = nc.dram_tensor("cosT", [JC, S], BF16, kind="ExternalInput").ap()
    shatT = nc.dram_tensor("shatT", [JC, S], BF16, kind="ExternalInput").ap()
    masks = nc.dram_tensor("masks", [128, 4 * 512], F32, kind="ExternalInput").ap()
    oT = nc.dram_tensor("oT", [HID, S], BF16, kind="ExternalOutput").ap()

    hsT_b = hsT.rearrange("(eb g p) s -> p eb g s", p=128, g=EB)   # [128, 8, EB, S]
    wqT_b = wqT.rearrange("(eb g p) j -> p eb g j", p=128, g=EB)   # [128, 8, EB, JC]
    wkT_b = wkT.rearrange("(eb g p) j -> p eb g j", p=128, g=EB)
    wvT_b = wvT.rearrange("(kt p) j -> p kt j", p=128)             # [128, 32, JC]
    woT_b = woT.rearrange("(jt p) e -> p jt e", p=128)             # [128, 4, HID]
    cosT_b = cosT.rearrange("(h p) s -> p h s", p=128)             # [128, 4, S]
    shatT_b = shatT.rearrange("(h p) s -> p h s", p=128)
    oT_b = oT.rearrange("(et p) s -> p et s", p=128)               # [128, 32, S]
    NB = KT // EB

    with tile.TileContext(nc) as tc:
        with (
            tc.tile_pool(name="wres", bufs=2) as wres,       # wv_m, wo_m resident
            tc.tile_pool(name="wst", bufs=4) as wst,         # wq/wk streamed chunks
            tc.tile_pool(name="trig", bufs=4) as trig,       # cos/shat per-sb stream
            tc.tile_pool(name="big", bufs=4) as big,         # rq/rk/v/attnT resident
            tc.tile_pool(name="hpool", bufs=2) as hpool,     # hs stream tiles
            tc.tile_pool(name="rtmp", bufs=4) as rtmp,       # RoPE f32 temps
            tc.tile_pool(name="expp", bufs=3) as expp,       # exp tiles bf16
            tc.tile_pool(name="small", bufs=1) as small,
            tc.tile_pool(name="zp", bufs=4) as zp,           # 1/z rows
            tc.tile_pool(name="outp", bufs=4) as outp,
            tc.tile_pool(name="ps", bufs=8, space="PSUM") as ps,
        ):
            rq = big.tile([128, HPC, S], BF16, tag="big", name="rq")
            rk = big.tile([128, HPC, S], BF16, tag="big", name="rk")
            wv_m = wres.tile([128, KT, JC], BF16, tag="w", name="wv")
            wo_m = wres.tile([128, HPC, HID], BF16, tag="w", name="wo")

            # ---- phase 1: QK projections + fused RoPE ----
            # wq/wk stream in eb-sized chunks alongside hs, so the PE can
            # start ~3 DMAs in; wv/wo resident loads trickle in one chunk
            # per sb and are ready by phase 2.
            for sb in range(SB):
                ss = slice(sb * 512, (sb + 1) * 512)
                ps_q = [ps.tile([128, 512], F32, tag="ps", name=f"psq{sb}_{i}")
                        for i in range(HPC)]
                ps_k = [ps.tile([128, 512], F32, tag="ps", name=f"psk{sb}_{i}")
                        for i in range(HPC)]
                cos_t = trig.tile([128, HPC, 512], BF16, tag="t", name=f"cos{sb}")
                shat_t = trig.tile([128, HPC, 512], BF16, tag="t", name=f"shat{sb}")
                for eb in range(NB):
                    # half-tile DMAs land on different queues: halves the
                    # transfer latency each chunk's first matmuls gate on
                    hst = hpool.tile([128, EB, 512], BF16, tag="h")
                    nc.sync.dma_start(hst[:, :EB // 2], hsT_b[:, eb, :EB // 2, ss])
                    wqs = wst.tile([128, EB, JC], BF16, tag="w", name=f"wqs{sb}_{eb}")
                    nc.sync.dma_start(wqs[:, :EB // 2], wqT_b[:, eb, :EB // 2])
                    wks = wst.tile([128, EB, JC], BF16, tag="w", name=f"wks{sb}_{eb}")
                    nc.sync.dma_start(wks[:, :EB // 2], wkT_b[:, eb, :EB // 2])
                    nc.sync.dma_start(hst[:, EB // 2:], hsT_b[:, eb, EB // 2:, ss])
                    nc.sync.dma_start(wqs[:, EB // 2:], wqT_b[:, eb, EB // 2:])
                    nc.sync.dma_start(wks[:, EB // 2:], wkT_b[:, eb, EB // 2:])
                    if eb == 0:
                        nc.sync.dma_start(cos_t, cosT_b[:, :, ss])
                        nc.sync.dma_start(shat_t, shatT_b[:, :, ss])
                    for g in range(EB):
                        e = eb * EB + g
                        for jt in range(HPC):
                            js = slice(jt * 128, (jt + 1) * 128)
                            nc.tensor.matmul(
                                ps_q[jt], wqs[:, g, js], hst[:, g],
                                start=(e == 0), stop=(e == KT - 1),
                            )
                            nc.tensor.matmul(
                                ps_k[jt], wks[:, g, js], hst[:, g],
                                start=(e == 0), stop=(e == KT - 1),
                            )
                # prefetch one quarter of wv/wo per sb
                pcs = slice(sb * 8, (sb + 1) * 8)
                nc.sync.dma_start(wv_m[:, pcs], wvT_b[:, pcs])
                nc.sync.dma_start(wo_m[:, :, sb * 1024:(sb + 1) * 1024],
                                  woT_b[:, :, sb * 1024:(sb + 1) * 1024])
                # RoPE: dst = qf*cos + rot_half(qf)*sin. shat holds the
                # HALF-SWAPPED signed sin (host-prepared), so both swap
                # multiplies have partition-ALIGNED inputs and only the
                # OUTPUT is partition-shifted (verified exact on HW).
                for ps_list, dst in ((ps_q, rq), (ps_k, rk)):
                    for jt in range(HPC):
                        p = ps_list[jt]
                        qf = rtmp.tile([128, 512], F32, tag="rt")
                        nc.scalar.copy(qf, p)   # frees the PSUM bank early
                        tmp = rtmp.tile([128, 512], F32, tag="rt")
                        nc.gpsimd.tensor_mul(
                            tmp[64:128], qf[0:64], shat_t[0:64, jt]
                        )
                        nc.vector.tensor_mul(
                            tmp[0:64], qf[64:128], shat_t[64:128, jt]
                        )
                        t2 = rtmp.tile([128, 512], F32, tag="rt")
                        nc.vector.tensor_mul(t2, qf, cos_t[:, jt])
                        with nc.allow_low_precision(reason="rope bf16 store"):
                            nc.vector.tensor_add(dst[:, jt, ss], t2, tmp)

            # ---- constants for attention (loaded behind phase 2's stream) ----
            masks_sb = small.tile([128, 4, 512], F32)
            nc.sync.dma_start(masks_sb, masks.rearrange("p (r j) -> p r j", r=4))
            onesf = small.tile([128, 1], F32)
            nc.vector.memset(onesf, 1.0)
            ones_bf = small.tile([128, 1], BF16)
            nc.vector.tensor_copy(ones_bf, onesf)
            onesf_r = small.tile([1, 128], F32)
            nc.vector.memset(onesf_r, 1.0)
            ones_row = small.tile([1, 128], F32R)
            nc.vector.tensor_copy(ones_row, onesf_r)

            # ---- phase 2: V projection (natural layout) ----
            v_m = big.tile([128, S // 128, JC], BF16, tag="big", name="v")
            for sb in range(SB):
                ss = slice(sb * 512, (sb + 1) * 512)
                ps_v = [ps.tile([128, 512], F32, tag="ps", name=f"psv{sb}_{i}")
                        for i in range(4)]
                for eb in range(NB):
                    hst = hpool.tile([128, EB, 512], BF16, tag="h")
                    nc.sync.dma_start(hst[:, :EB // 2], hsT_b[:, eb, :EB // 2, ss])
                    nc.sync.dma_start(hst[:, EB // 2:], hsT_b[:, eb, EB // 2:, ss])
                    for g in range(EB):
                        e = eb * EB + g
                        for t4 in range(4):
                            cs = slice(t4 * 128, (t4 + 1) * 128)
                            nc.tensor.matmul(
                                ps_v[t4], hst[:, g, cs], wv_m[:, e],
                                start=(e == 0), stop=(e == KT - 1),
                            )
                for t4 in range(4):
                    nc.scalar.copy(v_m[:, sb * 4 + t4], ps_v[t4])

            # ---- phase 3+4: attention (qb outer) + interleaved o_proj ----
            attnT = big.tile([128, HPC * SB, 512], BF16, tag="big", name="attnT")
            oo_flip = 0

            def emit_oproj(oqb, et):
                nonlocal oo_flip
                oqs = slice(oqb * 512, (oqb + 1) * 512)
                ps_oo = ps.tile([128, 512], F32, tag="ps",
                                name=f"poo{oqb}_{et}")
                for jt in range(HPC):
                    nc.tensor.matmul(
                        ps_oo, wo_m[:, jt, et * 128: (et + 1) * 128],
                        attnT[:, jt * SB + oqb],
                        start=(jt == 0), stop=(jt == HPC - 1),
                    )
                oo = outp.tile([128, 512], BF16, tag="oo", bufs=5)
                if oo_flip % 2 == 0:
                    nc.scalar.copy(oo, ps_oo)
                else:
                    nc.vector.tensor_copy(oo, ps_oo)
                oo_flip += 1
                nc.sync.dma_start(oT_b[:, et, oqs], oo)

            # o_proj for qb-1 is interleaved INTO qb's attention kt-loop:
            # its matmuls fill the exp-chain stalls, and attention's ACT
            # work hides under o_proj's solid PE blocks.
            pending = []
            for qb in range(SB):
                qs = slice(qb * 512, (qb + 1) * 512)
                nkt = 4 * qb + 4
                zrs = []
                pos_ = []
                for h in range(HPC):
                    i16 = h * SB + qb
                    ps_o = ps.tile([128, 512], F32, tag="ps", name=f"pso{qb}_{h}")
                    ps_z = ps.tile([1, 512], F32, tag="ps", name=f"psz{qb}_{h}")

                    def score_exp(kt):
                        # diagonal blocks: columns j < 128*r are fully
                        # masked -> skip them (w = valid width)
                        r = kt - 4 * qb
                        j0 = 128 * r if r > 0 else 0
                        w = 512 - j0
                        qsw_ = slice(qb * 512 + j0, (qb + 1) * 512)
                        ps_s = ps.tile([128, w], F32, tag="ps",
                                       name=f"pss{qb}_{h}_{kt}")
                        nc.tensor.matmul(
                            ps_s, rk[:, h, kt * 128: (kt + 1) * 128],
                            rq[:, h, qsw_], start=True, stop=True,
                        )
                        if r >= 0:
                            nc.vector.tensor_add(
                                ps_s, ps_s, masks_sb[:, r, j0:512]
                            )
                        ext = expp.tile([128, w], BF16, tag="exp")
                        nc.scalar.activation(
                            ext, ps_s, mybir.ActivationFunctionType.Exp,
                            scale=SCALE,
                        )
                        return ext, j0

                    # software pipeline: score/exp one kt ahead of z/AV, so
                    # the PE never waits on the mask+exp latency chain
                    nxt = score_exp(0)
                    for kt in range(nkt):
                        ext, j0 = nxt
                        if kt + 1 < nkt:
                            nxt = score_exp(kt + 1)
                        nc.tensor.matmul(
                            ps_z[:, j0:512], ones_bf, ext,
                            start=(kt == 0), stop=(kt == nkt - 1),
                        )
                        nc.tensor.matmul(
                            ps_o[:, j0:512],
                            v_m[:, kt, h * 128: (h + 1) * 128], ext,
                            start=(kt == 0), stop=(kt == nkt - 1),
                        )
                        if pending:
                            emit_oproj(*pending.pop(0))
                    # drain unnormalized rows (frees ps_o), 1/z row (frees ps_z)
                    nc.vector.tensor_copy(attnT[:, i16], ps_o)
                    zf = zp.tile([1, 512], F32, tag="zf", bufs=2)
                    nc.vector.reciprocal_approx_fast(zf, ps_z)
                    zr = zp.tile([1, 512], F32R, tag="zr")
                    nc.vector.tensor_copy(zr, zf)
                    zrs.append(zr)
                    pos_.append(i16)
                # normalize: broadcast 1/z across partitions on the PE
                for h in range(HPC):
                    zb = ps.tile([128, 512], F32, tag="ps", name=f"zb{qb}_{h}")
                    nc.tensor.matmul(zb, ones_row, zrs[h], start=True, stop=True)
                    i16 = pos_[h]
                    with nc.allow_low_precision(reason="attn normalize bf16"):
                        nc.vector.tensor_tensor(
                            attnT[:, i16], attnT[:, i16], zb,
                            op=mybir.AluOpType.mult,
                        )
                for job in pending:      # leftovers from qb-1
                    emit_oproj(*job)
                pending = [(qb, et) for et in range(KT)]
            for job in pending:              # final query block's o_proj
                emit_oproj(*job)

    nc.compile()
    return nc


def _get_nc():
    if "S" not in _CACHE:
        _CACHE["S"] = build()
    return _CACHE["S"]


def _causal_mask_templates():
    # masked (NEGM) iff 128*r + p > j for p in [0,128), j in [0,512)
    p = np.arange(128)[:, None]
    j = np.arange(512)[None, :]
    out = np.zeros((128, 4, 512), np.float32)
    for r in range(4):
        out[:, r, :] = np.where(128 * r + p > j, NEGM, 0.0).astype(np.float32)
    return np.ascontiguousarray(out.reshape(128, 4 * 512))


def _rope_cache_np():
    # mirrors reference._rope_cache in float32
    inv_freq = (1.0 / (BASE ** (np.arange(0, D, 2, dtype=np.float32) / np.float32(D)))).astype(np.float32)
    ratio = (MIN_R + (MAX_R - MIN_R) * (np.arange(H, dtype=np.float32) / np.float32(H))).astype(np.float32)
    t = (np.arange(S, dtype=np.float32)[None, :] / ratio[:, None]).astype(np.float32)
    freqs = (t[:, :, None] * inv_freq[None, None, :]).astype(np.float32)
    emb = np.concatenate([freqs, freqs], axis=-1)
    return np.cos(emb).astype(np.float32), np.sin(emb).astype(np.float32)


def _head_order(hs, Wq, Wk):
    """Exact head-outlier ordering from the last pre-RoPE attention row,
    computed in f64 on the host: srow_h = hs @ (Wk_h^T (Wq_h hs[-1]))."""
    hs64 = hs.astype(np.float64)
    q_last = hs64[-1] @ Wq.T.astype(np.float64)                 # [HID]
    Wk64 = Wk.astype(np.float64)
    Wall = np.empty((HID, H), np.float64)
    for h in range(H):
        rows = slice(h * D, (h + 1) * D)
        Wall[:, h] = Wk64[rows, :].T @ q_last[rows]
    srow = (hs64 @ Wall).T                                      # [H, S]
    sc = srow * SCALE
    m = sc.max(axis=-1, keepdims=True)
    e = np.exp(sc - m)
    aw = e / e.sum(axis=-1, keepdims=True)
    avg = aw.mean(axis=-1, keepdims=True)
    cnt = (aw > 3.0 * avg).sum(axis=-1)
    outlier = (-(cnt / np.float32(S))).astype(np.float32)
    return np.argsort(outlier, kind="stable")


def kernel(hidden_states, position_ids, Wq, Wk, Wv, Wo):
    hs = np.asarray(hidden_states, dtype=np.float32)[0]        # [S, HID]
    pos = np.asarray(position_ids).astype(np.int64)[0]         # [S]
    Wq = np.asarray(Wq, dtype=np.float32)
    Wk = np.asarray(Wk, dtype=np.float32)
    Wv = np.asarray(Wv, dtype=np.float32)
    Wo = np.asarray(Wo, dtype=np.float32)

    # ---- host: head order (exact control flow), permuted RoPE caches ----
    head_order = _head_order(hs, Wq, Wk)
    cos, sin = _rope_cache_np()
    cos_o = cos[head_order][:, pos, :]                         # [H, S, D]
    sin_o = sin[head_order][:, pos, :]
    masks = _causal_mask_templates()

    hsT = np.ascontiguousarray(hs.T).astype(NPBF16)            # [HID, S] bf16

    nc = _get_nc()
    in_maps = []
    for c in range(NCORES):
        rows = slice(c * JC, (c + 1) * JC)
        ct = np.ascontiguousarray(
            np.concatenate([cos_o[c * HPC + i].T for i in range(HPC)], axis=0)
        )  # [JC, S]
        # half-swapped signed sin: spre[0:64] = +sin[64:128],
        # spre[64:128] = -sin[0:64] (per head) — see RoPE comment in build()
        st = np.concatenate(
            [
                np.concatenate(
                    [sin_o[c * HPC + i].T[D // 2:], -sin_o[c * HPC + i].T[: D // 2]],
                    axis=0,
                )
                for i in range(HPC)
            ],
            axis=0,
        )
        in_maps.append(
            {
                "hsT": hsT,
                "wqT": np.ascontiguousarray(Wq[rows, :].T).astype(NPBF16),
                "wkT": np.ascontiguousarray(Wk[rows, :].T).astype(NPBF16),
                "wvT": np.ascontiguousarray(Wv[rows, :].T).astype(NPBF16),
                "woT": np.ascontiguousarray(Wo[:, rows].T).astype(NPBF16),
                "cosT": ct.astype(NPBF16),
                "shatT": np.ascontiguousarray(st).astype(NPBF16),
                "masks": masks,
            }
        )
    res = bass_utils.run_bass_kernel_spmd(
        nc, in_maps, core_ids=list(range(NCORES)), trace=TRACE
    )
    if TRACE:
        LAST_PROFILE["S"] = res

    # ---- host: unshard (sum o_proj partials) ----
    acc = np.zeros((HID, S), np.float64)
    for c in range(NCORES):
        acc += res.results[c]["oT"].astype(np.float64)
    return np.ascontiguousarray(acc.T)[None, :, :].astype(np.float32)


# revision 23
# speedup vs baseline: 1.0558x; 1.0091x over previous
"""MsPoE Llama attention on 8 TRN2 NeuronCores (tensor-parallel over heads).

Strategy (v2: single launch)
----------------------------
The reference's head-ordering statistic only needs the LAST pre-RoPE
attention row: srow_h = q_last_h . k_h[s]. By associativity,
srow_h = hs @ (Wk_h^T (Wq_h hs[-1])) — ~0.5 GFLOP, computed on the
host in float64 BEFORE launching (verified to reproduce the reference
head_order exactly: min margin to the 3*avg threshold is ~1e-5 vs
~6e-7 fp32-path noise). With head_order known up-front, the permuted
per-head RoPE cos/sin caches become plain inputs and the whole module
runs in ONE device launch with q/k/v resident in SBUF:

  Per core (4 heads), all matmul operands bf16 (1 PE cycle/row — the
  same rate as fp32r — but half the DMA/SBUF, and no f32->f32r
  conversion casts at all):

  1. QK pass: stream hsT once, accumulate q/k head-blocks in PSUM
     (8 banks), apply RoPE directly out of PSUM on DVE (+gpsimd for
     one swap-half) into resident rq/rk [128, 4, 2048] bf16 tiles.
     rotate_half's partition swap is two half-partition reads at
     offset 64/0; the sin sign flip is pre-applied on the host (shat).
  2. V pass: stream hsT again (wq+wk+wv cannot be resident at once),
     natural-layout V into resident v_m [128, 16, 512] bf16.
  3. Attention per (qb outer, head inner): scores^T = rk_chunk^T rq
     per 128-key tile, exp on ACT (bf16 out), softmax denominator via
     ones-matmul accumulation, unnormalized AV accumulation; 1/z is
     partition-broadcast with a tiny PE outer-product (no DRAM
     roundtrip); attnT normalized in place (DVE).
  4. o_proj for qb is emitted right after qb's heads finish, so its
     matmuls overlap the next qb's attention chain; partials oT
     [4096, 2048] f32 stream out per 512-column block.

  Host: sum the 8 o_proj partials (f64) -> [1, 2048, 4096].
"""

import os
import sys

import numpy as np

for _p in ("/opt/trn_rl_repo", "/root/.axon_site/_ro/trn_rl_repo"):
    if os.path.isdir(_p) and _p not in sys.path:
        sys.path.append(_p)

import concourse.bass as bass  # noqa: E402
import concourse.tile as tile  # noqa: E402
from concourse import bacc, mybir  # noqa: E402
from concourse import bass_utils  # noqa: E402

import ml_dtypes  # noqa: E402

F32 = mybir.dt.float32
BF16 = mybir.dt.bfloat16
F32R = mybir.dt.float32r
NPBF16 = ml_dtypes.bfloat16

B, S, HID, H, D = 1, 2048, 4096, 32, 128
NCORES, HPC = 8, 4          # cores, heads per core
JC = HPC * D                # 512: per-core projection width
KT = HID // 128             # 32 contraction tiles
SB = S // 512               # 4 sequence blocks
EB = 4                      # e-tiles per hs-stream DMA
BASE, MIN_R, MAX_R = 10000.0, 1.0, 3.0
SCALE = 1.0 / float(np.sqrt(D))
NEGM = -1.0e35              # additive causal mask value (exp -> 0)

_CACHE = {}
TRACE = False          # set True (e.g. from test.py) to profile the launch
LAST_PROFILE = {}      # filled with BassKernelResults when TRACE is on


def build():
    nc = bacc.Bacc("TRN2", target_bir_lowering=False, debug=False, num_devices=NCORES)
    hsT = nc.dram_tensor("hsT", [HID, S], BF16, kind="ExternalInput").ap()
    wqT = nc.dram_tensor("wqT", [HID, JC], BF16, kind="ExternalInput").ap()
    wkT = nc.dram_tensor("wkT", [HID, JC], BF16, kind="ExternalInput").ap()
    wvT = nc.dram_tensor("wvT", [HID, JC], BF16, kind="ExternalInput").ap()
    woT = nc.dram_tensor("woT", [JC, HID], BF16, kind="ExternalInput").ap()
    cosT = nc.dram_tensor("cosT", [JC, S], BF16, kind="ExternalInput").ap()
    shatT = nc.dram_tensor("shatT", [JC, S], BF16, kind="ExternalInput").ap()
    masks = nc.dram_tensor("masks", [128, 4 * 512], F32, kind="ExternalInput").ap()
    oT = nc.dram_tensor("oT", [HID, S], BF16, kind="ExternalOutput").ap()

    hsT_b = hsT.rearrange("(eb g p) s -> p eb g s", p=128, g=EB)   # [128, 8, EB, S]
    wqT_b = wqT.rearrange("(eb g p) j -> p eb g j", p=128, g=EB)   # [128, 8, EB, JC]
    wkT_b = wkT.rearrange("(eb g p) j -> p eb g j", p=128, g=EB)
    wvT_b = wvT.rearrange("(kt p) j -> p kt j", p=128)             # [128, 32, JC]
    woT_b = woT.rearrange("(jt p) e -> p jt e", p=128)             # [128, 4, HID]
    cosT_b = cosT.rearrange("(h p) s -> p h s", p=128)             # [128, 4, S]
    shatT_b = shatT.rearrange("(h p) s -> p h s", p=128)
    oT_b = oT.rearrange("(et p) s -> p et s", p=128)               # [128, 32, S]
    NB = KT // EB

    with tile.TileContext(nc) as tc:
        with (
            tc.tile_pool(name="wres", bufs=2) as wres,       # wv_m, wo_m resident
            tc.tile_pool(name="wst", bufs=4) as wst,         # wq/wk streamed chunks
            tc.tile_pool(name="trig", bufs=4) as trig,       # cos/shat per-sb stream
            tc.tile_pool(name="big", bufs=4) as big,         # rq/rk/v/attnT resident
            tc.tile_pool(name="hpool", bufs=2) as hpool,     # hs stream tiles
            tc.tile_pool(name="rtmp", bufs=4) as rtmp,       # RoPE f32 temps
            tc.tile_pool(name="expp", bufs=3) as expp,       # exp tiles bf16
            tc.tile_pool(name="small", bufs=1) as small,
            tc.tile_pool(name="zp", bufs=4) as zp,           # 1/z rows
            tc.tile_pool(name="outp", bufs=4) as outp,
            tc.tile_pool(name="ps", bufs=8, space="PSUM") as ps,
        ):
            rq = big.tile([128, HPC, S], BF16, tag="big", name="rq")
            rk = big.tile([128, HPC, S], BF16, tag="big", name="rk")
            wv_m = wres.tile([128, KT, JC], BF16, tag="w", name="wv")
            wo_m = wres.tile([128, HPC, HID], BF16, tag="w", name="wo")

            # ---- phase 1: QK projections + fused RoPE ----
            # wq/wk stream in eb-sized chunks alongside hs, so the PE can
            # start ~3 DMAs in; wv/wo resident loads trickle in one chunk
            # per sb and are ready by phase 2.
            for sb in range(SB):
                ss = slice(sb * 512, (sb + 1) * 512)
                ps_q = [ps.tile([128, 512], F32, tag="ps", name=f"psq{sb}_{i}")
                        for i in range(HPC)]
                ps_k = [ps.tile([128, 512], F32, tag="ps", name=f"psk{sb}_{i}")
                        for i in range(HPC)]
                cos_t = trig.tile([128, HPC, 512], BF16, tag="t", name=f"cos{sb}")
                shat_t = trig.tile([128, HPC, 512], BF16, tag="t", name=f"shat{sb}")
                for eb in range(NB):
                    # half-tile DMAs land on different queues: halves the
                    # transfer latency each chunk's first matmuls gate on
                    hst = hpool.tile([128, EB, 512], BF16, tag="h")
                    nc.sync.dma_start(hst[:, :EB // 2], hsT_b[:, eb, :EB // 2, ss])
                    wqs = wst.tile([128, EB, JC], BF16, tag="w", name=f"wqs{sb}_{eb}")
                    nc.sync.dma_start(wqs[:, :EB // 2], wqT_b[:, eb, :EB // 2])
                    wks = wst.tile([128, EB, JC], BF16, tag="w", name=f"wks{sb}_{eb}")
                    nc.sync.dma_start(wks[:, :EB // 2], wkT_b[:, eb, :EB // 2])
                    nc.sync.dma_start(hst[:, EB // 2:], hsT_b[:, eb, EB // 2:, ss])
                    nc.sync.dma_start(wqs[:, EB // 2:], wqT_b[:, eb, EB // 2:])
                    nc.sync.dma_start(wks[:, EB // 2:], wkT_b[:, eb, EB // 2:])
                    if eb == 0:
                        nc.sync.dma_start(cos_t, cosT_b[:, :, ss])
                        nc.sync.dma_start(shat_t, shatT_b[:, :, ss])
                    for g in range(EB):
                        e = eb * EB + g
                        for jt in range(HPC):
                            js = slice(jt * 128, (jt + 1) * 128)
                            nc.tensor.matmul(
                                ps_q[jt], wqs[:, g, js], hst[:, g],
                                start=(e == 0), stop=(e == KT - 1),
                            )
                            nc.tensor.matmul(
                                ps_k[jt], wks[:, g, js], hst[:, g],
                                start=(e == 0), stop=(e == KT - 1),
                            )
                # prefetch one quarter of wv/wo per sb
                pcs = slice(sb * 8, (sb + 1) * 8)
                nc.sync.dma_start(wv_m[:, pcs], wvT_b[:, pcs])
                nc.sync.dma_start(wo_m[:, :, sb * 1024:(sb + 1) * 1024],
                                  woT_b[:, :, sb * 1024:(sb + 1) * 1024])
                # RoPE: dst = qf*cos + rot_half(qf)*sin. shat holds the
                # HALF-SWAPPED signed sin (host-prepared), so both swap
                # multiplies have partition-ALIGNED inputs and only the
                # OUTPUT is partition-shifted (verified exact on HW).
                for ps_list, dst in ((ps_q, rq), (ps_k, rk)):
                    for jt in range(HPC):
                        p = ps_list[jt]
                        qf = rtmp.tile([128, 512], F32, tag="rt")
                        nc.scalar.copy(qf, p)   # frees the PSUM bank early
                        tmp = rtmp.tile([128, 512], F32, tag="rt")
                        nc.gpsimd.tensor_mul(
                            tmp[64:128], qf[0:64], shat_t[0:64, jt]
                        )
                        nc.vector.tensor_mul(
                            tmp[0:64], qf[64:128], shat_t[64:128, jt]
                        )
                        t2 = rtmp.tile([128, 512], F32, tag="rt")
                        nc.vector.tensor_mul(t2, qf, cos_t[:, jt])
                        with nc.allow_low_precision(reason="rope bf16 store"):
                            nc.vector.tensor_add(dst[:, jt, ss], t2, tmp)

            # ---- constants for attention (loaded behind phase 2's stream) ----
            masks_sb = small.tile([128, 4, 512], F32)
            nc.sync.dma_start(masks_sb, masks.rearrange("p (r j) -> p r j", r=4))
            onesf = small.tile([128, 1], F32)
            nc.vector.memset(onesf, 1.0)
            ones_bf = small.tile([128, 1], BF16)
            nc.vector.tensor_copy(ones_bf, onesf)
            onesf_r = small.tile([1, 128], F32)
            nc.vector.memset(onesf_r, 1.0)
            ones_row = small.tile([1, 128], F32R)
            nc.vector.tensor_copy(ones_row, onesf_r)

            # ---- phase 2: V projection (natural layout) ----
            v_m = big.tile([128, S // 128, JC], BF16, tag="big", name="v")
            for sb in range(SB):
                ss = slice(sb * 512, (sb + 1) * 512)
                ps_v = [ps.tile([128, 512], F32, tag="ps", name=f"psv{sb}_{i}")
                        for i in range(4)]
                for eb in range(NB):
                    hst = hpool.tile([128, EB, 512], BF16, tag="h")
                    nc.sync.dma_start(hst[:, :EB // 2], hsT_b[:, eb, :EB // 2, ss])
                    nc.sync.dma_start(hst[:, EB // 2:], hsT_b[:, eb, EB // 2:, ss])
                    for g in range(EB):
                        e = eb * EB + g
                        for t4 in range(4):
                            cs = slice(t4 * 128, (t4 + 1) * 128)
                            nc.tensor.matmul(
                                ps_v[t4], hst[:, g, cs], wv_m[:, e],
                                start=(e == 0), stop=(e == KT - 1),
                            )
                for t4 in range(4):
                    nc.scalar.copy(v_m[:, sb * 4 + t4], ps_v[t4])

            # ---- phase 3+4: attention (qb outer) + interleaved o_proj ----
            attnT = big.tile([128, HPC * SB, 512], BF16, tag="big", name="attnT")
            oo_flip = 0

            def emit_oproj(oqb, et):
                nonlocal oo_flip
                oqs = slice(oqb * 512, (oqb + 1) * 512)
                ps_oo = ps.tile([128, 512], F32, tag="ps",
                                name=f"poo{oqb}_{et}")
                for jt in range(HPC):
                    nc.tensor.matmul(
                        ps_oo, wo_m[:, jt, et * 128: (et + 1) * 128],
                        attnT[:, jt * SB + oqb],
                        start=(jt == 0), stop=(jt == HPC - 1),
                    )
                oo = outp.tile([128, 512], BF16, tag="oo", bufs=5)
                if oo_flip % 2 == 0:
                    nc.scalar.copy(oo, ps_oo)
                else:
                    nc.vector.tensor_copy(oo, ps_oo)
                oo_flip += 1
                m0 = oqb * 512
                nc.sync.dma_start(oT_b[:, et, m0:m0 + 256], oo[:, :256])
                nc.sync.dma_start(oT_b[:, et, m0 + 256:m0 + 512], oo[:, 256:])

            # o_proj for qb-1 is interleaved INTO qb's attention kt-loop:
            # its matmuls fill the exp-chain stalls, and attention's ACT
            # work hides under o_proj's solid PE blocks.
            pending = []
            for qb in range(SB):
                qs = slice(qb * 512, (qb + 1) * 512)
                nkt = 4 * qb + 4
                stride = max(1, (HPC * nkt) // KT)  # spread pops evenly
                it = 0
                zrs = []
                pos_ = []
                for h in range(HPC):
                    i16 = h * SB + qb
                    ps_o = ps.tile([128, 512], F32, tag="ps", name=f"pso{qb}_{h}")
                    ps_z = ps.tile([1, 512], F32, tag="ps", name=f"psz{qb}_{h}")

                    def score_exp(kt):
                        # diagonal blocks: columns j < 128*r are fully
                        # masked -> skip them (w = valid width)
                        r = kt - 4 * qb
                        j0 = 128 * r if r > 0 else 0
                        w = 512 - j0
                        qsw_ = slice(qb * 512 + j0, (qb + 1) * 512)
                        ps_s = ps.tile([128, w], F32, tag="ps",
                                       name=f"pss{qb}_{h}_{kt}")
                        nc.tensor.matmul(
                            ps_s, rk[:, h, kt * 128: (kt + 1) * 128],
                            rq[:, h, qsw_], start=True, stop=True,
                        )
                        if r >= 0:
                            nc.vector.tensor_add(
                                ps_s, ps_s, masks_sb[:, r, j0:512]
                            )
                        ext = expp.tile([128, w], BF16, tag="exp")
                        nc.scalar.activation(
                            ext, ps_s, mybir.ActivationFunctionType.Exp,
                            scale=SCALE,
                        )
                        return ext, j0

                    # software pipeline: score/exp one kt ahead of z/AV, so
                    # the PE never waits on the mask+exp latency chain
                    nxt = score_exp(0)
                    for kt in range(nkt):
                        ext, j0 = nxt
                        if kt + 1 < nkt:
                            nxt = score_exp(kt + 1)
                        nc.tensor.matmul(
                            ps_z[:, j0:512], ones_bf, ext,
                            start=(kt == 0), stop=(kt == nkt - 1),
                        )
                        nc.tensor.matmul(
                            ps_o[:, j0:512],
                            v_m[:, kt, h * 128: (h + 1) * 128], ext,
                            start=(kt == 0), stop=(kt == nkt - 1),
                        )
                        if pending and it % stride == 0:
                            emit_oproj(*pending.pop(0))
                        it += 1
                    # drain unnormalized rows (frees ps_o), 1/z row (frees ps_z)
                    nc.vector.tensor_copy(attnT[:, i16], ps_o)
                    zf = zp.tile([1, 512], F32, tag="zf", bufs=2)
                    nc.vector.reciprocal_approx_fast(zf, ps_z)
                    zr = zp.tile([1, 512], F32R, tag="zr")
                    nc.vector.tensor_copy(zr, zf)
                    zrs.append(zr)
                    pos_.append(i16)
                # normalize: broadcast 1/z across partitions on the PE
                for h in range(HPC):
                    zb = ps.tile([128, 512], F32, tag="ps", name=f"zb{qb}_{h}")
                    nc.tensor.matmul(zb, ones_row, zrs[h], start=True, stop=True)
                    i16 = pos_[h]
                    with nc.allow_low_precision(reason="attn normalize bf16"):
                        nc.vector.tensor_tensor(
                            attnT[:, i16], attnT[:, i16], zb,
                            op=mybir.AluOpType.mult,
                        )
                for job in pending:      # leftovers from qb-1
                    emit_oproj(*job)
                pending = [(qb, et) for et in range(KT)]
            for job in pending:              # final query block's o_proj
                emit_oproj(*job)

    nc.compile()
    return nc


def _get_nc():
    if "S" not in _CACHE:
        _CACHE["S"] = build()
    return _CACHE["S"]


def _causal_mask_templates():
    # masked (NEGM) iff 128*r + p > j for p in [0,128), j in [0,512)
    p = np.arange(128)[:, None]
    j = np.arange(512)[None, :]
    out = np.zeros((128, 4, 512), np.float32)
    for r in range(4):
        out[:, r, :] = np.where(128 * r + p > j, NEGM, 0.0).astype(np.float32)
    return np.ascontiguousarray(out.reshape(128, 4 * 512))


def _rope_cache_np():
    # mirrors reference._rope_cache in float32
    inv_freq = (1.0 / (BASE ** (np.arange(0, D, 2, dtype=np.float32) / np.float32(D)))).astype(np.float32)
    ratio = (MIN_R + (MAX_R - MIN_R) * (np.arange(H, dtype=np.float32) / np.float32(H))).astype(np.float32)
    t = (np.arange(S, dtype=np.float32)[None, :] / ratio[:, None]).astype(np.float32)
    freqs = (t[:, :, None] * inv_freq[None, None, :]).astype(np.float32)
    emb = np.concatenate([freqs, freqs], axis=-1)
    return np.cos(emb).astype(np.float32), np.sin(emb).astype(np.float32)


def _head_order(hs, Wq, Wk):
    """Exact head-outlier ordering from the last pre-RoPE attention row,
    computed in f64 on the host: srow_h = hs @ (Wk_h^T (Wq_h hs[-1]))."""
    hs64 = hs.astype(np.float64)
    q_last = hs64[-1] @ Wq.T.astype(np.float64)                 # [HID]
    Wk64 = Wk.astype(np.float64)
    Wall = np.empty((HID, H), np.float64)
    for h in range(H):
        rows = slice(h * D, (h + 1) * D)
        Wall[:, h] = Wk64[rows, :].T @ q_last[rows]
    srow = (hs64 @ Wall).T                                      # [H, S]
    sc = srow * SCALE
    m = sc.max(axis=-1, keepdims=True)
    e = np.exp(sc - m)
    aw = e / e.sum(axis=-1, keepdims=True)
    avg = aw.mean(axis=-1, keepdims=True)
    cnt = (aw > 3.0 * avg).sum(axis=-1)
    outlier = (-(cnt / np.float32(S))).astype(np.float32)
    return np.argsort(outlier, kind="stable")


def kernel(hidden_states, position_ids, Wq, Wk, Wv, Wo):
    hs = np.asarray(hidden_states, dtype=np.float32)[0]        # [S, HID]
    pos = np.asarray(position_ids).astype(np.int64)[0]         # [S]
    Wq = np.asarray(Wq, dtype=np.float32)
    Wk = np.asarray(Wk, dtype=np.float32)
    Wv = np.asarray(Wv, dtype=np.float32)
    Wo = np.asarray(Wo, dtype=np.float32)

    # ---- host: head order (exact control flow), permuted RoPE caches ----
    head_order = _head_order(hs, Wq, Wk)
    cos, sin = _rope_cache_np()
    cos_o = cos[head_order][:, pos, :]                         # [H, S, D]
    sin_o = sin[head_order][:, pos, :]
    masks = _causal_mask_templates()

    hsT = np.ascontiguousarray(hs.T).astype(NPBF16)            # [HID, S] bf16

    nc = _get_nc()
    in_maps = []
    for c in range(NCORES):
        rows = slice(c * JC, (c + 1) * JC)
        ct = np.ascontiguousarray(
            np.concatenate([cos_o[c * HPC + i].T for i in range(HPC)], axis=0)
        )  # [JC, S]
        # half-swapped signed sin: spre[0:64] = +sin[64:128],
        # spre[64:128] = -sin[0:64] (per head) — see RoPE comment in build()
        st = np.concatenate(
            [
                np.concatenate(
                    [sin_o[c * HPC + i].T[D // 2:], -sin_o[c * HPC + i].T[: D // 2]],
                    axis=0,
                )
                for i in range(HPC)
            ],
            axis=0,
        )
        in_maps.append(
            {
                "hsT": hsT,
                "wqT": np.ascontiguousarray(Wq[rows, :].T).astype(NPBF16),
                "wkT": np.ascontiguousarray(Wk[rows, :].T).astype(NPBF16),
                "wvT": np.ascontiguousarray(Wv[rows, :].T).astype(NPBF16),
                "woT": np.ascontiguousarray(Wo[:, rows].T).astype(NPBF16),
                "cosT": ct.astype(NPBF16),
                "shatT": np.ascontiguousarray(st).astype(NPBF16),
                "masks": masks,
            }
        )
    res = bass_utils.run_bass_kernel_spmd(
        nc, in_maps, core_ids=list(range(NCORES)), trace=TRACE
    )
    if TRACE:
        LAST_PROFILE["S"] = res

    # ---- host: unshard (sum o_proj partials) ----
    acc = np.zeros((HID, S), np.float64)
    for c in range(NCORES):
        acc += res.results[c]["oT"].astype(np.float64)
    return np.ascontiguousarray(acc.T)[None, :, :].astype(np.float32)
